# revision 27
# baseline (speedup 1.0000x reference)
"""Trainium2 Bass kernel for BertWithAdaThresholdLocContextPooling head.

Data-parallel over batch: 32 batches -> 8 NeuronCores x 4 batches.
Per core, everything is gather-based: only 8 mention rows of
sequence_output (entity 0) and 16*12 attention rows per batch are read
via indirect DMA -- the 400MB attention tensor is never fully read.

Math per batch b (faithful to the reference, including the hs-in-both-
extractors detail):
  hs  = logsumexp_m seq[pos[b,0,m]]                       [768]
  A_e = mean_m attention[:, pos[b,e,m], :]                [12, 512]
  w   = sum_h A_0 * A_1;  rs = (w @ seq[b]) / (sum(w) + 12e-5)
  x_f = tanh(W_f @ [hs | rs | ner_f | 1])   f in {head, tail}
  logits = W_bil @ vec(outer-per-group(x_head, x_tail)) + b_bil

All matmuls run with the contraction dim on SBUF partitions; activations
are kept feature-on-partition ("transposed") end to end so no on-device
transposes are needed. The grouped outer product is built with two
constant 16x128 replication matmuls per 128-feature chunk.
"""

import os

import numpy as np

import concourse.bass as bass
import concourse.tile as tile
from concourse import bacc, mybir
from concourse.bass_utils import run_bass_kernel_spmd

# problem dims
B, H, C, D = 32, 12, 512, 768
M = 8
EMB, BLK = 768, 8
NCLS, NER = 97, 6
OFFSET = 1
NCORES = 8
BL = B // NCORES            # batches per core
KIN = 2 * D + NER + 1       # 1543: [hs | rs | ner | 1]
KCH = 13                    # ceil(KIN / 128)
KLAST = KIN - 12 * 128      # 7 rows in the last chunk
GRP = EMB // BLK            # 96 bilinear groups
KP = EMB * BLK              # 6144
NT = KP // 128              # 48 bilinear chunks
F32 = mybir.dt.float32
BF16 = mybir.dt.bfloat16
I32 = mybir.dt.int32

# f32 const block [128, _CF_NCOL]: attention-path selectors + bias
_CF_SEL96 = 0       # [96,12]  mention-mean selector (1/M)
_CF_ONES128 = 12    # [1,128]
_CF_BBIL = 140      # [97,1]   bilinear bias
_CF_ONESC = 141     # [128,1]
_CF_NCOL = 142
# bf16 const block [128, _CB_NCOL]: mention-sum + head-sum selectors
_CB_SEL32 = 0       # [32,4]
_CB_ONES12 = 4      # [12,1]
_CB_NCOL = 5
# bf16 selector block [128, 512]: bilinear row replicators, [32,128] each,
# tiled at partition bases 0/32/64/96; cols = [A_v0 | A_v1 | B_v0 | B_v1]
_SAB_NCOL = 512

_CACHE = {}

LAST_EXEC_NS = None
LAST_RESULTS = None


def _build_nc():
    nc = bacc.Bacc("TRN2", target_bir_lowering=False, debug=False)

    seq_h = nc.dram_tensor("seq", [BL * C, D], BF16, kind="ExternalInput")
    attn_h = nc.dram_tensor("attn", [BL * H * C, C], F32, kind="ExternalInput")
    midx_h = nc.dram_tensor("midx", [BL * M, 1], I32, kind="ExternalInput")
    aidx_h = nc.dram_tensor("aidx", [M * H, 2 * BL], I32, kind="ExternalInput")
    nert_h = nc.dram_tensor("nert", [2 * (NER + 1), BL], BF16, kind="ExternalInput")
    wh_h = nc.dram_tensor("whT", [128, KCH * EMB], BF16, kind="ExternalInput")
    wt_h = nc.dram_tensor("wtT", [128, KCH * EMB], BF16, kind="ExternalInput")
    wb_h = nc.dram_tensor("wbT", [128, NT * NCLS], BF16, kind="ExternalInput")
    cstf_h = nc.dram_tensor("cstf", [128, _CF_NCOL], F32, kind="ExternalInput")
    cstb_h = nc.dram_tensor("cstb", [128, _CB_NCOL], BF16, kind="ExternalInput")
    sab_h = nc.dram_tensor("sab", [128, _SAB_NCOL], BF16, kind="ExternalInput")
    out_h = nc.dram_tensor("outT", [NCLS, BL], F32, kind="ExternalOutput")

    AF = mybir.ActivationFunctionType
    OP = mybir.AluOpType

    with tile.TileContext(nc) as tc:
        with (
            tc.tile_pool(name="w", bufs=1) as wp,
            tc.tile_pool(name="seqp", bufs=1) as sp,
            tc.tile_pool(name="g", bufs=2) as gp,
            tc.tile_pool(name="blp", bufs=3) as blp,
            tc.tile_pool(name="ps", bufs=8, space="PSUM") as pp,
        ):
            # small loads first on the scalar HWDGE ring; big weights on the
            # sync ring so they don't head-of-line-block the small/seq loads.
            midx_sb = wp.tile([BL * M, 1], I32)
            nc.scalar.dma_start(out=midx_sb[:], in_=midx_h[:])
            aidx_sb = wp.tile([M * H, 2 * BL], I32)
            nc.scalar.dma_start(out=aidx_sb[:], in_=aidx_h[:])
            cstf_sb = wp.tile([128, _CF_NCOL], F32)
            nc.scalar.dma_start(out=cstf_sb[:], in_=cstf_h[:])
            cstb_sb = wp.tile([128, _CB_NCOL], BF16)
            nc.scalar.dma_start(out=cstb_sb[:], in_=cstb_h[:])
            sab_sb = wp.tile([128, _SAB_NCOL], BF16)
            nc.scalar.dma_start(out=sab_sb[:], in_=sab_h[:])
            # two tiles so both matmul rhs operands sit at partition base 0
            nerh_sb = wp.tile([NER + 1, BL], BF16)
            nc.scalar.dma_start(out=nerh_sb[:], in_=nert_h[0 : NER + 1, :])
            nertl_sb = wp.tile([NER + 1, BL], BF16)
            nc.scalar.dma_start(out=nertl_sb[:], in_=nert_h[NER + 1 : 2 * (NER + 1), :])

            # big loads spread across the three DMA-capable queues: seq split
            # over SP+ACT (needed first), then whT/wbT on SP; wtT rides the
            # Pool queue behind the gathers (only the tail extractor needs it).
            seq_ts = []
            for i in range(4 * BL):
                seq_t = sp.tile([128, D], BF16, name=f"seq_t{i}")
                eng = nc.sync if i % 2 == 0 else nc.scalar
                eng.dma_start(out=seq_t[:], in_=seq_h[i * 128 : (i + 1) * 128, :])
                seq_ts.append(seq_t)
            wh_sb = wp.tile([128, KCH * EMB], BF16)
            nc.sync.dma_start(out=wh_sb[:], in_=wh_h[:])
            wb_sb = wp.tile([128, NT * NCLS], BF16)
            nc.sync.dma_start(out=wb_sb[:], in_=wb_h[:])

            sel96 = cstf_sb[0 : M * H, _CF_SEL96 : _CF_SEL96 + H]
            ones128 = cstf_sb[0:1, _CF_ONES128 : _CF_ONES128 + 128]
            bbil = cstf_sb[0:NCLS, _CF_BBIL : _CF_BBIL + 1]
            sel32 = cstb_sb[0 : BL * M, _CB_SEL32 : _CB_SEL32 + BL]
            ones12 = cstb_sb[0:H, _CB_ONES12 : _CB_ONES12 + 1]
            onescol = cstf_sb[0:128, _CF_ONESC : _CF_ONESC + 1]

            # inpT[:, c, :] = chunk c of [hs | rs] with features on partitions
            inpT = wp.tile([128, 12, BL], BF16)

            # ---- phase 1: mention gather + logsumexp -> inpT chunks 0..5
            g_ment = gp.tile([BL * M, D], BF16)
            nc.gpsimd.indirect_dma_start(
                out=g_ment[:],
                out_offset=None,
                in_=seq_h[:],
                in_offset=bass.IndirectOffsetOnAxis(ap=midx_sb[:, 0:1], axis=0),
            )
            expm = gp.tile([BL * M, D], BF16)
            nc.scalar.activation(expm[:], g_ment[:], AF.Exp)
            for c in range(6):
                lse_ps = pp.tile([128, BL], F32, tag="ps", name="lse_ps")
                nc.tensor.matmul(
                    out=lse_ps[:],
                    lhsT=expm[:, c * 128 : (c + 1) * 128],
                    rhs=sel32,
                    start=True,
                    stop=True,
                )
                nc.scalar.activation(inpT[:, c, :], lse_ps[:], AF.Ln)

            # ---- phase 2: attention gathers -> normalized context weights
            # wTn_sb[:, 4*b + c] = ht_att[b, c*128 + p] (c = seq chunk)
            wTn_sb = wp.tile([128, BL * 4], BF16)
            sraw = wp.tile([1, BL], F32)
            sden = wp.tile([1, BL], F32)
            srec = wp.tile([1, BL], F32)
            last_gather = None
            for b in range(BL):
                att_gs = []
                for e in range(2):
                    att_g = gp.tile([M * H, C], F32, tag="attg", name=f"att_g{e}")
                    col = b * 2 + e
                    last_gather = nc.gpsimd.indirect_dma_start(
                        out=att_g[:],
                        out_offset=None,
                        in_=attn_h[:],
                        in_offset=bass.IndirectOffsetOnAxis(
                            ap=aidx_sb[:, col : col + 1], axis=0
                        ),
                    )
                    att_gs.append(att_g)
                # transposed mention-mean: PT[c*128+p, h] = mean_m att[h, pos_m, c*128+p]
                # then w^T = sum_h PT0*PT1 fused on DVE
                wT_f = blp.tile([128, 4], F32, tag="wT_f")
                for c in range(4):
                    PT0 = pp.tile([128, H], F32, tag="ps", name="PT0")
                    PT1 = pp.tile([128, H], F32, tag="ps", name="PT1")
                    nc.tensor.matmul(
                        out=PT0[:], lhsT=att_gs[0][:, c * 128 : (c + 1) * 128],
                        rhs=sel96, start=True, stop=True,
                    )
                    nc.tensor.matmul(
                        out=PT1[:], lhsT=att_gs[1][:, c * 128 : (c + 1) * 128],
                        rhs=sel96, start=True, stop=True,
                    )
                    prodT = blp.tile([128, H], BF16, tag="prodT")
                    nc.vector.tensor_tensor_reduce(
                        out=prodT[:], in0=PT0[:], in1=PT1[:], scale=1.0,
                        scalar=0.0, op0=OP.mult, op1=OP.add,
                        accum_out=wT_f[:, c : c + 1],
                    )
                # s_b = sum(w) via PE (column sums then a 4-wide reduce)
                s_ps = pp.tile([1, 4], F32, tag="ps", name="s_ps")
                nc.tensor.matmul(
                    out=s_ps[:], lhsT=onescol, rhs=wT_f[:], start=True, stop=True
                )
                nc.vector.reduce_sum(
                    out=sraw[0:1, b : b + 1], in_=s_ps[:], axis=mybir.AxisListType.X
                )
                # ht = w / (sum(w) + H*1e-5); denominator folds the /H and /M^2
                nc.vector.tensor_scalar_add(
                    out=sden[0:1, b : b + 1],
                    in0=sraw[0:1, b : b + 1],
                    scalar1=float(H) * 1e-5,
                )
                nc.vector.reciprocal(out=srec[0:1, b : b + 1], in_=sden[0:1, b : b + 1])
                sb_ps = pp.tile([128, 1], F32, tag="ps", name="sb_ps")
                nc.tensor.matmul(
                    out=sb_ps[:],
                    lhsT=ones128,
                    rhs=srec[0:1, b : b + 1],
                    start=True,
                    stop=True,
                )
                nc.vector.tensor_tensor(
                    out=wTn_sb[:, b * 4 : (b + 1) * 4],
                    in0=wT_f[:],
                    in1=sb_ps[:, 0:1].to_broadcast([128, 4]),
                    op=OP.mult,
                )  # wTn_sb is bf16; DVE rounds on write

            # wtT rides the Pool queue but must not jump ahead of the gathers
            # (it is only needed by the tail extractor, much later).
            wt_sb = wp.tile([128, KCH * EMB], BF16)
            wt_dma = nc.gpsimd.dma_start(out=wt_sb[:], in_=wt_h[:])
            tile.add_dep_helper(
                wt_dma.ins, last_gather.ins, reason="wtT after att gathers"
            )

            # ---- phase 3: rs = ht_att @ seq  (resident seq tiles)
            rsT_ps = [
                pp.tile([128, BL], F32, tag="ps", name=f"rsT_ps{j}") for j in range(6)
            ]
            for b in range(BL):
                for c in range(4):
                    seq_t = seq_ts[b * 4 + c]
                    for j in range(6):
                        nc.tensor.matmul(
                            out=rsT_ps[j][:, b : b + 1],
                            lhsT=seq_t[:, j * 128 : (j + 1) * 128],
                            rhs=wTn_sb[:, (b * 4 + c) : (b * 4 + c) + 1],
                            start=(c == 0),
                            stop=(c == 3),
                        )
            for j in range(6):
                nc.vector.tensor_copy(inpT[:, 6 + j, :], rsT_ps[j][:])

            # ---- phase 4: extractors (features on partitions)
            # ex_fT[:, j, 0:4] = tanh head feats [j*128, (j+1)*128) for 4 batches
            # ex_fT[:, j, 4:8] = tanh tail feats
            ex_fT = wp.tile([128, 6, 2 * BL], BF16)
            for j in range(6):
                exh_ps = pp.tile([128, BL], F32, tag="ps", name="exh_ps")
                ext_ps = pp.tile([128, BL], F32, tag="ps", name="ext_ps")
                for c in range(KCH):
                    if c < 12:
                        lh = wh_sb[:, c * EMB + j * 128 : c * EMB + (j + 1) * 128]
                        lt = wt_sb[:, c * EMB + j * 128 : c * EMB + (j + 1) * 128]
                        rh = inpT[:, c, :]
                        rt = inpT[:, c, :]
                    else:
                        lh = wh_sb[0:KLAST, c * EMB + j * 128 : c * EMB + (j + 1) * 128]
                        lt = wt_sb[0:KLAST, c * EMB + j * 128 : c * EMB + (j + 1) * 128]
                        rh = nerh_sb[:]
                        rt = nertl_sb[:]
                    nc.tensor.matmul(
                        out=exh_ps[:], lhsT=lh, rhs=rh, start=(c == 0), stop=(c == KCH - 1)
                    )
                    nc.tensor.matmul(
                        out=ext_ps[:], lhsT=lt, rhs=rt, start=(c == 0), stop=(c == KCH - 1)
                    )
                nc.scalar.activation(ex_fT[:, j, 0:BL], exh_ps[:], AF.Tanh)
                nc.scalar.activation(ex_fT[:, j, BL : 2 * BL], ext_ps[:], AF.Tanh)

            # ---- phase 5: grouped bilinear + output matmul
            logit_ps = pp.tile([NCLS, BL], F32, tag="ps", name="logit_ps")
            for tg in range(NT // 4):
                psA4 = pp.tile([128, 4 * 2 * BL], F32, tag="ps", name="psA4")
                psB4 = pp.tile([128, 4 * 2 * BL], F32, tag="ps", name="psB4")
                for i in range(4):
                    t = tg * 4 + i
                    j6, r = t // 8, t % 8
                    base, v = 32 * (r // 2), r % 2
                    src = ex_fT[base : base + 32, j6, :]
                    selA = sab_sb[base : base + 32, v * 128 : (v + 1) * 128]
                    selB = sab_sb[base : base + 32, (2 + v) * 128 : (3 + v) * 128]
                    nc.tensor.matmul(
                        out=psA4[:, i * 8 : (i + 1) * 8], lhsT=selA, rhs=src,
                        start=True, stop=True, tile_position=(base, 0),
                    )
                    nc.tensor.matmul(
                        out=psB4[:, i * 8 : (i + 1) * 8], lhsT=selB, rhs=src,
                        start=True, stop=True, tile_position=(base, 0),
                    )
                blT4 = blp.tile([128, 4, BL], BF16, tag="blT4")
                nc.vector.tensor_tensor(
                    out=blT4[:, :, :],
                    in0=psA4[:].rearrange("p (i c) -> p i c", c=8)[:, :, 0:BL],
                    in1=psB4[:].rearrange("p (i c) -> p i c", c=8)[:, :, BL : 2 * BL],
                    op=OP.mult,
                )
                for i in range(4):
                    t = tg * 4 + i
                    nc.tensor.matmul(
                        out=logit_ps[:],
                        lhsT=wb_sb[:, t * NCLS : (t + 1) * NCLS],
                        rhs=blT4[:, i, :],
                        start=(t == 0),
                        stop=(t == NT - 1),
                    )
            logitsT_sb = wp.tile([NCLS, BL], F32)
            nc.vector.tensor_scalar_add(out=logitsT_sb[:], in0=logit_ps[:], scalar1=bbil)
            nc.scalar.dma_start(out=out_h[:], in_=logitsT_sb[:])

    nc.compile()
    return nc


def _bf16(x):
    import ml_dtypes

    return np.ascontiguousarray(np.asarray(x).astype(ml_dtypes.bfloat16))


def _weights_prep(W_head, b_head, W_tail, b_tail, W_bil, b_bil):
    """Host-side weight packing (transposed + chunk-interleaved + bias rows)."""

    def pack_ext(Wf, bf):
        ext = np.zeros((KCH * 128, EMB), np.float32)
        ext[: 2 * D + NER] = Wf.T.astype(np.float32)
        ext[2 * D + NER] = bf.astype(np.float32)
        return _bf16(
            ext.reshape(KCH, 128, EMB).transpose(1, 0, 2).reshape(128, KCH * EMB)
        )

    whT = pack_ext(W_head, b_head)
    wtT = pack_ext(W_tail, b_tail)

    wbe = np.asarray(W_bil, np.float32).T  # [KP, NCLS]
    wbT = _bf16(wbe.reshape(NT, 128, NCLS).transpose(1, 0, 2).reshape(128, NT * NCLS))

    cstf = np.zeros((128, _CF_NCOL), np.float32)
    for m in range(M):
        for h in range(H):
            cstf[m * H + h, _CF_SEL96 + h] = 1.0 / M
    cstf[0, _CF_ONES128 : _CF_ONES128 + 128] = 1.0
    cstf[0:NCLS, _CF_BBIL] = b_bil.astype(np.float32)
    cstf[0:128, _CF_ONESC] = 1.0

    cstb = np.zeros((128, _CB_NCOL), np.float32)
    for b in range(BL):
        for m in range(M):
            cstb[b * M + m, _CB_SEL32 + b] = 1.0
    cstb[0:H, _CB_ONES12] = 1.0

    # [32,128] replicators, variant v covers bl-chunk rows 16v..16v+15,
    # tiled at every 32-partition base so lhsT/rhs partition bases match
    sab32 = np.zeros((32, _SAB_NCOL), np.float32)
    p = np.arange(128)
    srcA = (p // 64) * 8 + (p % 64) // 8
    srcB = (p // 64) * 8 + (p % 8)
    for v in range(2):
        sab32[16 * v + srcA, v * 128 + p] = 1.0
        sab32[16 * v + srcB, (2 + v) * 128 + p] = 1.0
    sab = np.tile(sab32, (4, 1))
    return whT, wtT, wbT, cstf, _bf16(cstb), _bf16(sab)


def _make_in_maps(inputs):
    seq = np.ascontiguousarray(np.asarray(inputs["sequence_output"], np.float32))
    att = np.ascontiguousarray(np.asarray(inputs["attention"], np.float32))
    ner = np.asarray(inputs["ner_tags"], np.float32)
    ep = np.asarray(inputs["entity_pos"]).astype(np.int64)
    pos = ep + OFFSET  # [B, 2, M]

    whT, wtT, wbT, cstf, cstb, sab = _weights_prep(
        np.asarray(inputs["W_head"]),
        np.asarray(inputs["b_head"]),
        np.asarray(inputs["W_tail"]),
        np.asarray(inputs["b_tail"]),
        np.asarray(inputs["W_bil"]),
        np.asarray(inputs["b_bil"]),
    )

    in_maps = []
    for k in range(NCORES):
        b0 = k * BL
        seq_k = _bf16(seq[b0 : b0 + BL].reshape(BL * C, D))
        att_k = att[b0 : b0 + BL].reshape(BL * H * C, C)

        midx = np.zeros((BL * M, 1), np.int32)
        for b in range(BL):
            midx[b * M : (b + 1) * M, 0] = b * C + pos[b0 + b, 0, :]

        aidx = np.zeros((M * H, 2 * BL), np.int32)
        mh_h = np.tile(np.arange(H), M)  # row p = m*H + h -> h
        mh_m = np.repeat(np.arange(M), H)  # -> m
        for b in range(BL):
            for e in range(2):
                aidx[:, b * 2 + e] = (b * H + mh_h) * C + pos[b0 + b, e, mh_m]

        nert = np.zeros((2 * (NER + 1), BL), np.float32)
        nert[0:NER] = ner[b0 : b0 + BL, 0, :].T
        nert[NER] = 1.0
        nert[NER + 1 : 2 * NER + 1] = ner[b0 : b0 + BL, 1, :].T
        nert[2 * NER + 1] = 1.0

        in_maps.append(
            {
                "seq": seq_k,
                "attn": np.ascontiguousarray(att_k),
                "midx": midx,
                "aidx": aidx,
                "nert": _bf16(nert),
                "whT": whT,
                "wtT": wtT,
                "wbT": wbT,
                "cstf": cstf,
                "cstb": cstb,
                "sab": sab,
            }
        )
    return in_maps


def _get_nc():
    if "nc" not in _CACHE:
        _CACHE["nc"] = _build_nc()
    return _CACHE["nc"]


def kernel(**inputs):
    global LAST_EXEC_NS, LAST_RESULTS
    nc = _get_nc()
    in_maps = _make_in_maps(inputs)
    trace = bool(int(os.environ.get("BASS_KERNEL_TRACE", "0")))
    try:
        res = run_bass_kernel_spmd(
            nc, in_maps, core_ids=list(range(NCORES)), trace=trace
        )
    except Exception:
        if not trace:
            raise
        # tracing infra unavailable in this environment -- run untraced
        res = run_bass_kernel_spmd(
            nc, in_maps, core_ids=list(range(NCORES)), trace=False
        )
    LAST_EXEC_NS = res.exec_time_ns
    LAST_RESULTS = res
    out = np.zeros((B, NCLS), np.float32)
    for k in range(NCORES):
        out[k * BL : (k + 1) * BL] = np.asarray(res.results[k]["outT"]).T
    return out


# revision 30
# speedup vs baseline: 1.1152x; 1.1152x over previous
"""Trainium2 Bass kernel for BertWithAdaThresholdLocContextPooling head.

Data-parallel over batch: 32 batches -> 8 NeuronCores x 4 batches.
Per core, everything is gather-based: only 8 mention rows of
sequence_output (entity 0) and 16*12 attention rows per batch are read
via indirect DMA -- the 400MB attention tensor is never fully read.

Math per batch b (faithful to the reference, including the hs-in-both-
extractors detail):
  hs  = logsumexp_m seq[pos[b,0,m]]                       [768]
  A_e = mean_m attention[:, pos[b,e,m], :]                [12, 512]
  w   = sum_h A_0 * A_1;  rs = (w @ seq[b]) / (sum(w) + 12e-5)
  x_f = tanh(W_f @ [hs | rs | ner_f | 1])   f in {head, tail}
  logits = W_bil @ vec(outer-per-group(x_head, x_tail)) + b_bil

All matmuls run with the contraction dim on SBUF partitions; activations
are kept feature-on-partition ("transposed") end to end so no on-device
transposes are needed. The grouped outer product is built with two
constant 16x128 replication matmuls per 128-feature chunk.
"""

import os

import numpy as np

import concourse.bass as bass
import concourse.tile as tile
from concourse import bacc, mybir
from concourse.bass_utils import run_bass_kernel_spmd

# problem dims
B, H, C, D = 32, 12, 512, 768
M = 8
EMB, BLK = 768, 8
NCLS, NER = 97, 6
OFFSET = 1
NCORES = 8
BL = B // NCORES            # batches per core
KIN = 2 * D + NER + 1       # 1543: [hs | rs | ner | 1]
KCH = 13                    # ceil(KIN / 128)
KLAST = KIN - 12 * 128      # 7 rows in the last chunk
GRP = EMB // BLK            # 96 bilinear groups
KP = EMB * BLK              # 6144
NT = KP // 128              # 48 bilinear chunks
F32 = mybir.dt.float32
BF16 = mybir.dt.bfloat16
I32 = mybir.dt.int32

# f32 const block [128, _CF_NCOL]: attention-path selectors + bias
_CF_SEL96 = 0       # [96,12]  mention-mean selector (1/M)
_CF_ONES128 = 12    # [1,128]
_CF_BBIL = 140      # [97,1]   bilinear bias
_CF_ONESC = 141     # [128,1]
_CF_NCOL = 142
# bf16 const block [128, _CB_NCOL]: mention-sum + head-sum selectors
_CB_SEL32 = 0       # [32,4]
_CB_ONES12 = 4      # [12,1]
_CB_NCOL = 5
# bf16 selector block [128, 512]: bilinear row replicators, [32,128] each,
# tiled at partition bases 0/32/64/96; cols = [A_v0 | A_v1 | B_v0 | B_v1]
_SAB_NCOL = 512

_CACHE = {}

LAST_EXEC_NS = None
LAST_RESULTS = None


def _build_nc():
    nc = bacc.Bacc("TRN2", target_bir_lowering=False, debug=False)

    seq_h = nc.dram_tensor("seq", [BL * C, D], BF16, kind="ExternalInput")
    attn_h = nc.dram_tensor("attn", [BL * H * C, C], F32, kind="ExternalInput")
    midx_h = nc.dram_tensor("midx", [BL * M, 1], I32, kind="ExternalInput")
    aidx_h = nc.dram_tensor("aidx", [M * H, 2 * BL], I32, kind="ExternalInput")
    nert_h = nc.dram_tensor("nert", [2 * (NER + 1), BL], BF16, kind="ExternalInput")
    wh_h = nc.dram_tensor("whT", [128, KCH * EMB], BF16, kind="ExternalInput")
    wt_h = nc.dram_tensor("wtT", [128, KCH * EMB], BF16, kind="ExternalInput")
    wb_h = nc.dram_tensor("wbT", [128, NT * NCLS], BF16, kind="ExternalInput")
    cstf_h = nc.dram_tensor("cstf", [128, _CF_NCOL], F32, kind="ExternalInput")
    cstb_h = nc.dram_tensor("cstb", [128, _CB_NCOL], BF16, kind="ExternalInput")
    sab_h = nc.dram_tensor("sab", [128, _SAB_NCOL], BF16, kind="ExternalInput")
    out_h = nc.dram_tensor("outT", [NCLS, BL], F32, kind="ExternalOutput")

    AF = mybir.ActivationFunctionType
    OP = mybir.AluOpType

    with tile.TileContext(nc) as tc:
        with (
            tc.tile_pool(name="w", bufs=1) as wp,
            tc.tile_pool(name="seqp", bufs=1) as sp,
            tc.tile_pool(name="g", bufs=2) as gp,
            tc.tile_pool(name="blp", bufs=3) as blp,
            tc.tile_pool(name="ps", bufs=8, space="PSUM") as pp,
        ):
            # small loads first on the scalar HWDGE ring; big weights on the
            # sync ring so they don't head-of-line-block the small/seq loads.
            midx_sb = wp.tile([BL * M, 1], I32)
            nc.scalar.dma_start(out=midx_sb[:], in_=midx_h[:])
            aidx_sb = wp.tile([M * H, 2 * BL], I32)
            nc.scalar.dma_start(out=aidx_sb[:], in_=aidx_h[:])
            cstf_sb = wp.tile([128, _CF_NCOL], F32)
            nc.scalar.dma_start(out=cstf_sb[:], in_=cstf_h[:])
            cstb_sb = wp.tile([128, _CB_NCOL], BF16)
            nc.scalar.dma_start(out=cstb_sb[:], in_=cstb_h[:])
            sab_sb = wp.tile([128, _SAB_NCOL], BF16)
            nc.scalar.dma_start(out=sab_sb[:], in_=sab_h[:])
            # two tiles so both matmul rhs operands sit at partition base 0
            nerh_sb = wp.tile([NER + 1, BL], BF16)
            nc.scalar.dma_start(out=nerh_sb[:], in_=nert_h[0 : NER + 1, :])
            nertl_sb = wp.tile([NER + 1, BL], BF16)
            nc.scalar.dma_start(out=nertl_sb[:], in_=nert_h[NER + 1 : 2 * (NER + 1), :])

            sel96 = cstf_sb[0 : M * H, _CF_SEL96 : _CF_SEL96 + H]
            ones128 = cstf_sb[0:1, _CF_ONES128 : _CF_ONES128 + 128]
            bbil = cstf_sb[0:NCLS, _CF_BBIL : _CF_BBIL + 1]
            sel32 = cstb_sb[0 : BL * M, _CB_SEL32 : _CB_SEL32 + BL]
            ones12 = cstb_sb[0:H, _CB_ONES12 : _CB_ONES12 + 1]
            onescol = cstf_sb[0:128, _CF_ONESC : _CF_ONESC + 1]

            # inpT[:, c, :] = chunk c of [hs | rs] with features on partitions
            inpT = wp.tile([128, 12, BL], BF16)

            # ---- phase 1: mention gather + logsumexp -> inpT chunks 0..5
            g_ment = gp.tile([BL * M, D], BF16)
            nc.gpsimd.indirect_dma_start(
                out=g_ment[:],
                out_offset=None,
                in_=seq_h[:],
                in_offset=bass.IndirectOffsetOnAxis(ap=midx_sb[:, 0:1], axis=0),
            )
            expm = gp.tile([BL * M, D], BF16)
            nc.scalar.activation(expm[:], g_ment[:], AF.Exp)
            # all 6 chunk sums share one single-bank PSUM tile
            lse_ps = pp.tile([128, 6 * BL], F32, tag="ps", name="lse_ps")
            for c in range(6):
                nc.tensor.matmul(
                    out=lse_ps[:, c * BL : (c + 1) * BL],
                    lhsT=expm[:, c * 128 : (c + 1) * 128],
                    rhs=sel32,
                    start=True,
                    stop=True,
                )
            nc.scalar.activation(inpT[:, 0:6, :], lse_ps[:], AF.Ln)

            # big loads spread across the three DMA-capable queues: seq split
            # over SP+ACT (needed first), whT on SP, wbT on ACT; wtT rides
            # the Pool queue behind the gathers.
            seq_ts = []
            for i in range(4 * BL):
                seq_t = sp.tile([128, D], BF16, name=f"seq_t{i}")
                eng = nc.sync if i % 2 == 0 else nc.scalar
                eng.dma_start(out=seq_t[:], in_=seq_h[i * 128 : (i + 1) * 128, :])
                seq_ts.append(seq_t)
            wh_sb = wp.tile([128, KCH * EMB], BF16)
            nc.sync.dma_start(out=wh_sb[:], in_=wh_h[:])
            wb_sb = wp.tile([128, NT * NCLS], BF16)
            nc.scalar.dma_start(out=wb_sb[:], in_=wb_h[:])

            # ---- phase 2: attention gathers -> normalized context weights
            # wTn_sb[:, 4*b + c] = ht_att[b, c*128 + p] (c = seq chunk)
            wTn_sb = wp.tile([128, BL * 4], BF16)
            sraw = wp.tile([1, BL], F32)
            sden = wp.tile([1, BL], F32)
            srec = wp.tile([1, BL], F32)
            last_gather = None
            for b in range(BL):
                att_gs = []
                for e in range(2):
                    att_g = gp.tile([M * H, C], F32, tag="attg", name=f"att_g{e}")
                    col = b * 2 + e
                    last_gather = nc.gpsimd.indirect_dma_start(
                        out=att_g[:],
                        out_offset=None,
                        in_=attn_h[:],
                        in_offset=bass.IndirectOffsetOnAxis(
                            ap=aidx_sb[:, col : col + 1], axis=0
                        ),
                    )
                    att_gs.append(att_g)
                # transposed mention-mean: PT[c*128+p, h] = mean_m att[h, pos_m, c*128+p]
                # then w^T = sum_h PT0*PT1 fused on DVE
                wT_f = blp.tile([128, 4], F32, tag="wT_f")
                PT = pp.tile([128, 4, 2, H], F32, tag="ps", name="PT")
                for c in range(4):
                    for e in range(2):
                        nc.tensor.matmul(
                            out=PT[:, c, e, :],
                            lhsT=att_gs[e][:, c * 128 : (c + 1) * 128],
                            rhs=sel96, start=True, stop=True,
                        )
                for c in range(4):
                    prodT = blp.tile([128, H], BF16, tag="prodT")
                    nc.vector.tensor_tensor_reduce(
                        out=prodT[:], in0=PT[:, c, 0, :], in1=PT[:, c, 1, :],
                        scale=1.0, scalar=0.0, op0=OP.mult, op1=OP.add,
                        accum_out=wT_f[:, c : c + 1],
                    )
                # s_b = sum(w) via PE (column sums then a 4-wide reduce)
                s_ps = pp.tile([1, 4], F32, tag="ps", name="s_ps")
                nc.tensor.matmul(
                    out=s_ps[:], lhsT=onescol, rhs=wT_f[:], start=True, stop=True
                )
                nc.vector.reduce_sum(
                    out=sraw[0:1, b : b + 1], in_=s_ps[:], axis=mybir.AxisListType.X
                )
                # ht = w / (sum(w) + H*1e-5); denominator folds the /H and /M^2
                nc.vector.tensor_scalar_add(
                    out=sden[0:1, b : b + 1],
                    in0=sraw[0:1, b : b + 1],
                    scalar1=float(H) * 1e-5,
                )
                nc.vector.reciprocal(out=srec[0:1, b : b + 1], in_=sden[0:1, b : b + 1])
                sb_ps = pp.tile([128, 1], F32, tag="ps", name="sb_ps")
                nc.tensor.matmul(
                    out=sb_ps[:],
                    lhsT=ones128,
                    rhs=srec[0:1, b : b + 1],
                    start=True,
                    stop=True,
                )
                nc.vector.tensor_tensor(
                    out=wTn_sb[:, b * 4 : (b + 1) * 4],
                    in0=wT_f[:],
                    in1=sb_ps[:, 0:1].to_broadcast([128, 4]),
                    op=OP.mult,
                )  # wTn_sb is bf16; DVE rounds on write

            # wtT rides the Pool queue but must not jump ahead of the gathers
            # (it is only needed by the tail extractor, much later).
            wt_sb = wp.tile([128, KCH * EMB], BF16)
            wt_dma = nc.gpsimd.dma_start(out=wt_sb[:], in_=wt_h[:])
            tile.add_dep_helper(
                wt_dma.ins, last_gather.ins, reason="wtT after att gathers"
            )

            # ---- phase 3: rs = ht_att @ seq  (resident seq tiles)
            # one 2KB bank; groups must be sequential (start zeroes lazily by
            # whole region), so keep each (b,j) group's 4 matmuls consecutive
            rsT_ps = pp.tile([128, 6, BL], F32, tag="ps", name="rsT_ps")
            for b in range(BL):
                for j in range(6):
                    for c in range(4):
                        nc.tensor.matmul(
                            out=rsT_ps[:, j, b : b + 1],
                            lhsT=seq_ts[b * 4 + c][:, j * 128 : (j + 1) * 128],
                            rhs=wTn_sb[:, (b * 4 + c) : (b * 4 + c) + 1],
                            start=(c == 0),
                            stop=(c == 3),
                        )
            nc.vector.tensor_copy(inpT[:, 6:12, :], rsT_ps[:, :, :])

            # ---- phase 4: extractors (features on partitions)
            # ex_fT[:, j, 0:4] = tanh head feats [j*128, (j+1)*128) for 4 batches
            # ex_fT[:, j, 4:8] = tanh tail feats
            ex_fT = wp.tile([128, 6, 2 * BL], BF16)
            for j in range(6):
                exh_ps = pp.tile([128, BL], F32, tag="ps", name="exh_ps")
                ext_ps = pp.tile([128, BL], F32, tag="ps", name="ext_ps")
                for c in range(KCH):
                    if c < 12:
                        lh = wh_sb[:, c * EMB + j * 128 : c * EMB + (j + 1) * 128]
                        lt = wt_sb[:, c * EMB + j * 128 : c * EMB + (j + 1) * 128]
                        rh = inpT[:, c, :]
                        rt = inpT[:, c, :]
                    else:
                        lh = wh_sb[0:KLAST, c * EMB + j * 128 : c * EMB + (j + 1) * 128]
                        lt = wt_sb[0:KLAST, c * EMB + j * 128 : c * EMB + (j + 1) * 128]
                        rh = nerh_sb[:]
                        rt = nertl_sb[:]
                    nc.tensor.matmul(
                        out=exh_ps[:], lhsT=lh, rhs=rh, start=(c == 0), stop=(c == KCH - 1)
                    )
                    nc.tensor.matmul(
                        out=ext_ps[:], lhsT=lt, rhs=rt, start=(c == 0), stop=(c == KCH - 1)
                    )
                nc.scalar.activation(ex_fT[:, j, 0:BL], exh_ps[:], AF.Tanh)
                nc.scalar.activation(ex_fT[:, j, BL : 2 * BL], ext_ps[:], AF.Tanh)

            # ---- phase 5: grouped bilinear + output matmul
            logit_ps = pp.tile([NCLS, BL], F32, tag="ps", name="logit_ps")
            for tg in range(NT // 4):
                psA4 = pp.tile([128, 4 * 2 * BL], F32, tag="ps", name="psA4")
                psB4 = pp.tile([128, 4 * 2 * BL], F32, tag="ps", name="psB4")
                for i in range(4):
                    t = tg * 4 + i
                    j6, r = t // 8, t % 8
                    base, v = 32 * (r // 2), r % 2
                    src = ex_fT[base : base + 32, j6, :]
                    selA = sab_sb[base : base + 32, v * 128 : (v + 1) * 128]
                    selB = sab_sb[base : base + 32, (2 + v) * 128 : (3 + v) * 128]
                    nc.tensor.matmul(
                        out=psA4[:, i * 8 : (i + 1) * 8], lhsT=selA, rhs=src,
                        start=True, stop=True, tile_position=(base, 0),
                    )
                    nc.tensor.matmul(
                        out=psB4[:, i * 8 : (i + 1) * 8], lhsT=selB, rhs=src,
                        start=True, stop=True, tile_position=(base, 0),
                    )
                blT4 = blp.tile([128, 4, BL], BF16, tag="blT4")
                nc.vector.tensor_tensor(
                    out=blT4[:, :, :],
                    in0=psA4[:].rearrange("p (i c) -> p i c", c=8)[:, :, 0:BL],
                    in1=psB4[:].rearrange("p (i c) -> p i c", c=8)[:, :, BL : 2 * BL],
                    op=OP.mult,
                )
                for i in range(4):
                    t = tg * 4 + i
                    nc.tensor.matmul(
                        out=logit_ps[:],
                        lhsT=wb_sb[:, t * NCLS : (t + 1) * NCLS],
                        rhs=blT4[:, i, :],
                        start=(t == 0),
                        stop=(t == NT - 1),
                    )
            logitsT_sb = wp.tile([NCLS, BL], F32)
            nc.vector.tensor_scalar_add(out=logitsT_sb[:], in0=logit_ps[:], scalar1=bbil)
            nc.scalar.dma_start(out=out_h[:], in_=logitsT_sb[:])

    nc.compile()
    return nc


def _bf16(x):
    import ml_dtypes

    return np.ascontiguousarray(np.asarray(x).astype(ml_dtypes.bfloat16))


def _weights_prep(W_head, b_head, W_tail, b_tail, W_bil, b_bil):
    """Host-side weight packing (transposed + chunk-interleaved + bias rows)."""

    def pack_ext(Wf, bf):
        ext = np.zeros((KCH * 128, EMB), np.float32)
        ext[: 2 * D + NER] = Wf.T.astype(np.float32)
        ext[2 * D + NER] = bf.astype(np.float32)
        return _bf16(
            ext.reshape(KCH, 128, EMB).transpose(1, 0, 2).reshape(128, KCH * EMB)
        )

    whT = pack_ext(W_head, b_head)
    wtT = pack_ext(W_tail, b_tail)

    wbe = np.asarray(W_bil, np.float32).T  # [KP, NCLS]
    wbT = _bf16(wbe.reshape(NT, 128, NCLS).transpose(1, 0, 2).reshape(128, NT * NCLS))

    cstf = np.zeros((128, _CF_NCOL), np.float32)
    for m in range(M):
        for h in range(H):
            cstf[m * H + h, _CF_SEL96 + h] = 1.0 / M
    cstf[0, _CF_ONES128 : _CF_ONES128 + 128] = 1.0
    cstf[0:NCLS, _CF_BBIL] = b_bil.astype(np.float32)
    cstf[0:128, _CF_ONESC] = 1.0

    cstb = np.zeros((128, _CB_NCOL), np.float32)
    for b in range(BL):
        for m in range(M):
            cstb[b * M + m, _CB_SEL32 + b] = 1.0
    cstb[0:H, _CB_ONES12] = 1.0

    # [32,128] replicators, variant v covers bl-chunk rows 16v..16v+15,
    # tiled at every 32-partition base so lhsT/rhs partition bases match
    sab32 = np.zeros((32, _SAB_NCOL), np.float32)
    p = np.arange(128)
    srcA = (p // 64) * 8 + (p % 64) // 8
    srcB = (p // 64) * 8 + (p % 8)
    for v in range(2):
        sab32[16 * v + srcA, v * 128 + p] = 1.0
        sab32[16 * v + srcB, (2 + v) * 128 + p] = 1.0
    sab = np.tile(sab32, (4, 1))
    return whT, wtT, wbT, cstf, _bf16(cstb), _bf16(sab)


def _make_in_maps(inputs):
    seq = np.ascontiguousarray(np.asarray(inputs["sequence_output"], np.float32))
    att = np.ascontiguousarray(np.asarray(inputs["attention"], np.float32))
    ner = np.asarray(inputs["ner_tags"], np.float32)
    ep = np.asarray(inputs["entity_pos"]).astype(np.int64)
    pos = ep + OFFSET  # [B, 2, M]

    whT, wtT, wbT, cstf, cstb, sab = _weights_prep(
        np.asarray(inputs["W_head"]),
        np.asarray(inputs["b_head"]),
        np.asarray(inputs["W_tail"]),
        np.asarray(inputs["b_tail"]),
        np.asarray(inputs["W_bil"]),
        np.asarray(inputs["b_bil"]),
    )

    in_maps = []
    for k in range(NCORES):
        b0 = k * BL
        seq_k = _bf16(seq[b0 : b0 + BL].reshape(BL * C, D))
        att_k = att[b0 : b0 + BL].reshape(BL * H * C, C)

        midx = np.zeros((BL * M, 1), np.int32)
        for b in range(BL):
            midx[b * M : (b + 1) * M, 0] = b * C + pos[b0 + b, 0, :]

        aidx = np.zeros((M * H, 2 * BL), np.int32)
        mh_h = np.tile(np.arange(H), M)  # row p = m*H + h -> h
        mh_m = np.repeat(np.arange(M), H)  # -> m
        for b in range(BL):
            for e in range(2):
                aidx[:, b * 2 + e] = (b * H + mh_h) * C + pos[b0 + b, e, mh_m]

        nert = np.zeros((2 * (NER + 1), BL), np.float32)
        nert[0:NER] = ner[b0 : b0 + BL, 0, :].T
        nert[NER] = 1.0
        nert[NER + 1 : 2 * NER + 1] = ner[b0 : b0 + BL, 1, :].T
        nert[2 * NER + 1] = 1.0

        in_maps.append(
            {
                "seq": seq_k,
                "attn": np.ascontiguousarray(att_k),
                "midx": midx,
                "aidx": aidx,
                "nert": _bf16(nert),
                "whT": whT,
                "wtT": wtT,
                "wbT": wbT,
                "cstf": cstf,
                "cstb": cstb,
                "sab": sab,
            }
        )
    return in_maps


def _get_nc():
    if "nc" not in _CACHE:
        _CACHE["nc"] = _build_nc()
    return _CACHE["nc"]


def kernel(**inputs):
    global LAST_EXEC_NS, LAST_RESULTS
    nc = _get_nc()
    in_maps = _make_in_maps(inputs)
    trace = bool(int(os.environ.get("BASS_KERNEL_TRACE", "0")))
    try:
        res = run_bass_kernel_spmd(
            nc, in_maps, core_ids=list(range(NCORES)), trace=trace
        )
    except Exception:
        if not trace:
            raise
        # tracing infra unavailable in this environment -- run untraced
        res = run_bass_kernel_spmd(
            nc, in_maps, core_ids=list(range(NCORES)), trace=False
        )
    LAST_EXEC_NS = res.exec_time_ns
    LAST_RESULTS = res
    out = np.zeros((B, NCLS), np.float32)
    for k in range(NCORES):
        out[k * BL : (k + 1) * BL] = np.asarray(res.results[k]["outT"]).T
    return out


# revision 48
# speedup vs baseline: 1.7561x; 1.5747x over previous
"""Trainium2 Bass kernel for BertWithAdaThresholdLocContextPooling head.

Data-parallel over batch: 32 batches -> 8 NeuronCores x 4 batches.
Per core, everything is gather-based: only 8 mention rows of
sequence_output (entity 0) and 16*12 attention rows per batch are read
via indirect DMA -- the 400MB attention tensor is never fully read.

Math per batch b (faithful to the reference, including the hs-in-both-
extractors detail):
  hs  = logsumexp_m seq[pos[b,0,m]]                       [768]
  A_e = mean_m attention[:, pos[b,e,m], :]                [12, 512]
  w   = sum_h A_0 * A_1;  rs = (w @ seq[b]) / (sum(w) + 12e-5)
  x_f = tanh(W_f @ [hs | rs | ner_f | 1])   f in {head, tail}
  logits = W_bil @ vec(outer-per-group(x_head, x_tail)) + b_bil

All matmuls run with the contraction dim on SBUF partitions; activations
are kept feature-on-partition ("transposed") end to end so no on-device
transposes are needed. The grouped outer product is built with constant
32x128 replication matmuls. Weights/activations are bf16 (f32 PSUM
accumulation); the attention path stays f32 until the head product.
"""

import os

import numpy as np

import concourse.bass as bass
import concourse.tile as tile
from concourse import bacc, mybir
from concourse.bass_utils import run_bass_kernel_spmd

# problem dims
B, H, C, D = 32, 12, 512, 768
M = 8
EMB, BLK = 768, 8
NCLS, NER = 97, 6
OFFSET = 1
NCORES = 8
BL = B // NCORES            # batches per core
KIN = 2 * D + NER + 1       # 1543: [hs | rs | ner | 1]
KCH = 13                    # ceil(KIN / 128)
KLAST = KIN - 12 * 128      # 7 rows in the last chunk
KP = EMB * BLK              # 6144
NT = KP // 128              # 48 bilinear chunks
F32 = mybir.dt.float32
BF16 = mybir.dt.bfloat16
I32 = mybir.dt.int32

# f32 const block [128, _CF_NCOL]
_CF_SEL96 = 0       # [96,12]  mention-mean selector (1/M)
_CF_ONES128 = 12    # [1,128]
_CF_BBIL = 140      # [97,1]   bilinear bias
_CF_ONESC = 141     # [128,1]
_CF_NCOL = 142
# bf16 const block [128, _CB_NCOL]
_CB_SEL32 = 0       # [32,4]   mention->batch sum selector
_CB_ONES12 = 4      # [12,1]
_CB_NERH = 5        # [7,4]    [ner0 | 1] per batch (extractor last chunk)
_CB_NERT = 9        # [7,4]    [ner1 | 1]
_CB_SAB = 13        # 8 x [64,128] bilinear row replicators, tiled at bases
_CB_NCOL = 13 + 8 * 128   # 0/64; cols = [A_v0..A_v3 | B_v0..B_v3]

_CACHE = {}

LAST_EXEC_NS = None
LAST_RESULTS = None


def _build_nc():
    nc = bacc.Bacc("TRN2", target_bir_lowering=False, debug=False)

    seq_h = nc.dram_tensor("seq", [BL * C, D], BF16, kind="ExternalInput")
    attn_h = nc.dram_tensor("attn", [BL * H * C, C], F32, kind="ExternalInput")
    idx_h = nc.dram_tensor("idx", [M * H, 1 + 2 * BL], I32, kind="ExternalInput")
    wh_h = nc.dram_tensor("whT", [128, KCH * EMB], BF16, kind="ExternalInput")
    wt_h = nc.dram_tensor("wtT", [128, KCH * EMB], BF16, kind="ExternalInput")
    wb_h = nc.dram_tensor("wbT", [128, NT * NCLS], BF16, kind="ExternalInput")
    cstf_h = nc.dram_tensor("cstf", [128, _CF_NCOL], F32, kind="ExternalInput")
    cstb_h = nc.dram_tensor("cstb", [128, _CB_NCOL], BF16, kind="ExternalInput")
    out_h = nc.dram_tensor("outT", [NCLS, BL], F32, kind="ExternalOutput")

    AF = mybir.ActivationFunctionType
    OP = mybir.AluOpType

    with tile.TileContext(nc) as tc:
        with (
            tc.tile_pool(name="w", bufs=1) as wp,
            tc.tile_pool(name="seqp", bufs=1) as sp,
            tc.tile_pool(name="attp", bufs=1) as ap,
            tc.tile_pool(name="g", bufs=2) as gp,
            tc.tile_pool(name="blp", bufs=3) as blp,
            tc.tile_pool(name="ps", bufs=8, space="PSUM") as pp,
        ):
            # --- consolidated small loads first on the ACT queue
            idx_sb = wp.tile([M * H, 1 + 2 * BL], I32)
            nc.sync.dma_start(out=idx_sb[:], in_=idx_h[:])
            cstf_sb = wp.tile([128, _CF_NCOL], F32)
            nc.sync.dma_start(out=cstf_sb[:], in_=cstf_h[:])
            cstb_sb = wp.tile([128, _CB_NCOL], BF16)
            nc.sync.dma_start(out=cstb_sb[:], in_=cstb_h[:])

            sel96 = cstf_sb[0 : M * H, _CF_SEL96 : _CF_SEL96 + H]
            ones128 = cstf_sb[0:1, _CF_ONES128 : _CF_ONES128 + 128]
            bbil = cstf_sb[0:NCLS, _CF_BBIL : _CF_BBIL + 1]
            onescol = cstf_sb[0:128, _CF_ONESC : _CF_ONESC + 1]
            sel32 = cstb_sb[0 : BL * M, _CB_SEL32 : _CB_SEL32 + BL]
            ones12 = cstb_sb[0:H, _CB_ONES12 : _CB_ONES12 + 1]
            nerh = cstb_sb[0:KLAST, _CB_NERH : _CB_NERH + BL]
            nert = cstb_sb[0:KLAST, _CB_NERT : _CB_NERT + BL]

            # batch-3 seq tiles early on SP (ACT's tail seq would gate rs)
            seq_sp = {}
            for i in range(12, 16):
                seq_t = sp.tile([128, D], BF16, name=f"seq_t{i}")
                nc.sync.dma_start(out=seq_t[:], in_=seq_h[i * 128 : (i + 1) * 128, :])
                seq_sp[i] = seq_t

            # inpT[:, c, :] = chunk c of [hs | rs] with features on partitions
            inpT = wp.tile([128, 12, BL], BF16)

            # ---- phase 1: mention gather + logsumexp -> inpT chunks 0..5
            g_ment = gp.tile([BL * M, D], BF16)
            nc.gpsimd.indirect_dma_start(
                out=g_ment[:],
                out_offset=None,
                in_=seq_h[:],
                in_offset=bass.IndirectOffsetOnAxis(ap=idx_sb[0 : BL * M, 0:1], axis=0),
            )
            # all 8 attention-row gathers up front on the Pool queue,
            # into resident tiles (no slot recycling stalls)
            att_gs = {}
            for b in range(BL):
                for e in range(2):
                    col = 1 + b * 2 + e
                    att_g = ap.tile([M * H, C], F32, name=f"att_g{b}_{e}")
                    nc.gpsimd.indirect_dma_start(
                        out=att_g[:],
                        out_offset=None,
                        in_=attn_h[:],
                        in_offset=bass.IndirectOffsetOnAxis(
                            ap=idx_sb[:, col : col + 1], axis=0
                        ),
                    )
                    att_gs[b, e] = att_g

            seq_ts = []
            for i in range(8):
                seq_t = sp.tile([128, D], BF16, name=f"seq_t{i}")
                nc.scalar.dma_start(out=seq_t[:], in_=seq_h[i * 128 : (i + 1) * 128, :])
                seq_ts.append(seq_t)

            expm = gp.tile([BL * M, D], BF16)
            nc.scalar.activation(expm[:], g_ment[:], AF.Exp)
            # all 6 chunk sums share one single-bank PSUM tile
            lse_ps = pp.tile([128, 6 * BL], F32, tag="ps", name="lse_ps")
            for c in range(6):
                nc.tensor.matmul(
                    out=lse_ps[:, c * BL : (c + 1) * BL],
                    lhsT=expm[:, c * 128 : (c + 1) * 128],
                    rhs=sel32,
                    start=True,
                    stop=True,
                )
            nc.scalar.activation(inpT[:, 0:6, :], lse_ps[:], AF.Ln)

            # --- remaining seq on ACT; whT on SP; wbT behind the gathers on
            # Pool; wtT split in thirds across all three DMA queues
            for i in range(8, 12):
                seq_t = sp.tile([128, D], BF16, name=f"seq_t{i}")
                nc.scalar.dma_start(out=seq_t[:], in_=seq_h[i * 128 : (i + 1) * 128, :])
                seq_ts.append(seq_t)
            seq_ts.extend(seq_sp[i] for i in range(12, 16))
            wh_sb = wp.tile([128, KCH * EMB], BF16)
            nc.sync.dma_start(out=wh_sb[:], in_=wh_h[:])
            # wtT pieces sized so all three queues finish it ~together;
            # on Pool it goes ahead of wbT (needed earlier)
            wt_sb = wp.tile([128, KCH * EMB], BF16)
            nc.gpsimd.dma_start(out=wt_sb[:, 5888:], in_=wt_h[:, 5888:])
            nc.sync.dma_start(out=wt_sb[:, 0:2176], in_=wt_h[:, 0:2176])
            nc.scalar.dma_start(out=wt_sb[:, 2176:5888], in_=wt_h[:, 2176:5888])
            wb_sb = wp.tile([128, NT * NCLS], BF16)
            nc.gpsimd.dma_start(out=wb_sb[:], in_=wb_h[:])

            # ---- phase 2: attention means -> normalized context weights
            # wTn_sb[:, 4*b + c] = ht_att[b, c*128 + p] (c = seq chunk)
            wTn_sb = wp.tile([128, BL * 4], BF16)
            sraw = wp.tile([1, BL], F32)
            sden = wp.tile([1, BL], F32)
            srec = wp.tile([1, BL], F32)
            for b in range(BL):
                # transposed mention-mean: PT[c*128+p, e, h] then
                # w^T[:, c] = sum_h PT0*PT1 fused on DVE
                wT_f = blp.tile([128, 4], F32, tag="wT_f")
                PT = pp.tile([128, 4, 2, H], F32, tag="ps", name="PT")
                for c in range(4):
                    for e in range(2):
                        nc.tensor.matmul(
                            out=PT[:, c, e, :],
                            lhsT=att_gs[b, e][:, c * 128 : (c + 1) * 128],
                            rhs=sel96, start=True, stop=True,
                        )
                # DVE may read only one PSUM operand: stage the e=0 half in SBUF
                pte0 = blp.tile([128, 4, H], F32, tag="pte0")
                nc.vector.tensor_copy(pte0[:, :, :], PT[:, :, 0, :])
                prodT = blp.tile([128, 4, H], F32, tag="prodT")
                nc.vector.tensor_tensor(
                    out=prodT[:, :, :], in0=pte0[:, :, :], in1=PT[:, :, 1, :],
                    op=OP.mult,
                )
                nc.vector.reduce_sum(
                    out=wT_f[:], in_=prodT[:, :, :], axis=mybir.AxisListType.X
                )
                # s_b = sum(w) via PE (column sums then a 4-wide reduce)
                s_ps = pp.tile([1, 4], F32, tag="ps", name="s_ps")
                nc.tensor.matmul(
                    out=s_ps[:], lhsT=onescol, rhs=wT_f[:], start=True, stop=True
                )
                nc.vector.reduce_sum(
                    out=sraw[0:1, b : b + 1], in_=s_ps[:], axis=mybir.AxisListType.X
                )
                # ht = w / (sum(w) + H*1e-5); denominator folds the /H and /M^2
                nc.vector.tensor_scalar_add(
                    out=sden[0:1, b : b + 1],
                    in0=sraw[0:1, b : b + 1],
                    scalar1=float(H) * 1e-5,
                )
                nc.vector.reciprocal(out=srec[0:1, b : b + 1], in_=sden[0:1, b : b + 1])
                sb_ps = pp.tile([128, 1], F32, tag="ps", name="sb_ps")
                nc.tensor.matmul(
                    out=sb_ps[:],
                    lhsT=ones128,
                    rhs=srec[0:1, b : b + 1],
                    start=True,
                    stop=True,
                )
                nc.vector.tensor_tensor(
                    out=wTn_sb[:, b * 4 : (b + 1) * 4],
                    in0=wT_f[:],
                    in1=sb_ps[:, 0:1].to_broadcast([128, 4]),
                    op=OP.mult,
                )  # wTn_sb is bf16; DVE rounds on write

            # ---- phase 3: rs = ht_att @ seq  (resident seq tiles)
            # three banks, alternating groups so same-bank sem delays overlap;
            # each (b,j) group's 4 matmuls stay consecutive (lazy zero region)
            rsT = [
                pp.tile([128, 2, BL], F32, tag="ps", name=f"rsT{k}") for k in range(3)
            ]
            for b in range(BL):
                for j in range(6):
                    for c in range(4):
                        nc.tensor.matmul(
                            out=rsT[j % 3][:, j // 3, b : b + 1],
                            lhsT=seq_ts[b * 4 + c][:, j * 128 : (j + 1) * 128],
                            rhs=wTn_sb[:, (b * 4 + c) : (b * 4 + c) + 1],
                            start=(c == 0),
                            stop=(c == 3),
                        )
            for k in range(3):
                nc.vector.tensor_copy(inpT[:, 6 + k : 12 : 3, :], rsT[k][:, :, :])

            # ---- phase 4: extractors (features on partitions)
            # ex_fT[:, j, 0:4] = tanh head feats [j*128, (j+1)*128) for 4
            # batches; ex_fT[:, j, 4:8] = tanh tail feats
            ex_fT = wp.tile([128, 6, 2 * BL], BF16)
            corder = list(range(6, 12)) + [12] + list(range(6))  # rs first
            for j in range(6):
                # head + tail share one bank as two sequential groups
                ex_ps = pp.tile([128, 2 * BL], F32, tag="ps", name="ex_ps")
                for half, (w_sb, rner) in enumerate(((wh_sb, nerh), (wt_sb, nert))):
                    for ci, c in enumerate(corder):
                        if c < 12:
                            l = w_sb[:, c * EMB + j * 128 : c * EMB + (j + 1) * 128]
                            r = inpT[:, c, :]
                        else:
                            l = w_sb[0:KLAST, c * EMB + j * 128 : c * EMB + (j + 1) * 128]
                            r = rner
                        nc.tensor.matmul(
                            out=ex_ps[:, half * BL : (half + 1) * BL], lhsT=l, rhs=r,
                            start=(ci == 0), stop=(ci == KCH - 1),
                        )
                nc.scalar.activation(ex_fT[:, j, :], ex_ps[:], AF.Tanh)

            # ---- phase 5: grouped bilinear + output matmul
            logit_ps = pp.tile([NCLS, BL], F32, tag="ps", name="logit_ps")
            for tg in range(NT // 8):
                psA4 = pp.tile([128, 8 * 2 * BL], F32, tag="ps", name="psA4")
                psB4 = pp.tile([128, 8 * 2 * BL], F32, tag="ps", name="psB4")
                for i in range(8):
                    t = tg * 8 + i
                    j6, r = t // 8, t % 8
                    base, v = 64 * (r // 4), r % 4
                    src = ex_fT[base : base + 64, j6, :]
                    selA = cstb_sb[base : base + 64,
                                   _CB_SAB + v * 128 : _CB_SAB + (v + 1) * 128]
                    selB = cstb_sb[base : base + 64,
                                   _CB_SAB + (4 + v) * 128 : _CB_SAB + (5 + v) * 128]
                    nc.tensor.matmul(
                        out=psA4[:, i * 8 : (i + 1) * 8], lhsT=selA, rhs=src,
                        start=True, stop=True,
                    )
                    nc.tensor.matmul(
                        out=psB4[:, i * 8 : (i + 1) * 8], lhsT=selB, rhs=src,
                        start=True, stop=True,
                    )
                # stage psA4 in SBUF (single-PSUM-operand rule)
                psA_sb = blp.tile([128, 8 * 2 * BL], F32, tag="psA_sb")
                nc.vector.tensor_copy(psA_sb[:], psA4[:])
                blT4 = blp.tile([128, 8, BL], BF16, tag="blT4")
                nc.vector.tensor_tensor(
                    out=blT4[:, :, :],
                    in0=psA_sb[:].rearrange("p (i c) -> p i c", c=8)[:, :, 0:BL],
                    in1=psB4[:].rearrange("p (i c) -> p i c", c=8)[:, :, BL : 2 * BL],
                    op=OP.mult,
                )
                for i in range(8):
                    t = tg * 8 + i
                    nc.tensor.matmul(
                        out=logit_ps[:],
                        lhsT=wb_sb[:, t * NCLS : (t + 1) * NCLS],
                        rhs=blT4[:, i, :],
                        start=(t == 0),
                        stop=(t == NT - 1),
                    )
            logitsT_sb = wp.tile([NCLS, BL], F32)
            nc.vector.tensor_scalar_add(out=logitsT_sb[:], in0=logit_ps[:], scalar1=bbil)
            nc.sync.dma_start(out=out_h[:], in_=logitsT_sb[:])

    nc.compile()
    return nc


def _bf16(x):
    import ml_dtypes

    return np.ascontiguousarray(np.asarray(x).astype(ml_dtypes.bfloat16))


def _weights_prep(W_head, b_head, W_tail, b_tail, W_bil, b_bil):
    """Host-side weight packing (transposed + chunk-interleaved + bias rows)."""

    def pack_ext(Wf, bf):
        ext = np.zeros((KCH * 128, EMB), np.float32)
        ext[: 2 * D + NER] = Wf.T.astype(np.float32)
        ext[2 * D + NER] = bf.astype(np.float32)
        return _bf16(
            ext.reshape(KCH, 128, EMB).transpose(1, 0, 2).reshape(128, KCH * EMB)
        )

    whT = pack_ext(W_head, b_head)
    wtT = pack_ext(W_tail, b_tail)

    wbe = np.asarray(W_bil, np.float32).T  # [KP, NCLS]
    wbT = _bf16(wbe.reshape(NT, 128, NCLS).transpose(1, 0, 2).reshape(128, NT * NCLS))

    cstf = np.zeros((128, _CF_NCOL), np.float32)
    for m in range(M):
        for h in range(H):
            cstf[m * H + h, _CF_SEL96 + h] = 1.0 / M
    cstf[0, _CF_ONES128 : _CF_ONES128 + 128] = 1.0
    cstf[0:NCLS, _CF_BBIL] = b_bil.astype(np.float32)
    cstf[0:128, _CF_ONESC] = 1.0
    return whT, wtT, wbT, cstf


def _cstb_prep(ner_slice):
    """Per-core bf16 const block: selectors + ner columns + bilinear sab."""
    cstb = np.zeros((128, _CB_NCOL), np.float32)
    for b in range(BL):
        for m in range(M):
            cstb[b * M + m, _CB_SEL32 + b] = 1.0
    cstb[0:H, _CB_ONES12] = 1.0
    cstb[0:NER, _CB_NERH : _CB_NERH + BL] = ner_slice[:, 0, :].T
    cstb[NER, _CB_NERH : _CB_NERH + BL] = 1.0
    cstb[0:NER, _CB_NERT : _CB_NERT + BL] = ner_slice[:, 1, :].T
    cstb[NER, _CB_NERT : _CB_NERT + BL] = 1.0
    # [64,128] replicators, variant v covers bl-chunk rows 16v..16v+15,
    # tiled at bases 0/64 so lhsT/rhs partition bases match
    p = np.arange(128)
    srcA = (p // 64) * 8 + (p % 64) // 8
    srcB = (p // 64) * 8 + (p % 8)
    sab64 = np.zeros((64, 8 * 128), np.float32)
    for v in range(4):
        sab64[16 * v + srcA, v * 128 + p] = 1.0
        sab64[16 * v + srcB, (4 + v) * 128 + p] = 1.0
    cstb[:, _CB_SAB:] = np.tile(sab64, (2, 1))
    return _bf16(cstb)


def _make_in_maps(inputs):
    seq = np.asarray(inputs["sequence_output"], np.float32)
    att = np.ascontiguousarray(np.asarray(inputs["attention"], np.float32))
    ner = np.asarray(inputs["ner_tags"], np.float32)
    ep = np.asarray(inputs["entity_pos"]).astype(np.int64)
    pos = ep + OFFSET  # [B, 2, M]

    whT, wtT, wbT, cstf = _weights_prep(
        np.asarray(inputs["W_head"]),
        np.asarray(inputs["b_head"]),
        np.asarray(inputs["W_tail"]),
        np.asarray(inputs["b_tail"]),
        np.asarray(inputs["W_bil"]),
        np.asarray(inputs["b_bil"]),
    )

    in_maps = []
    mh_h = np.tile(np.arange(H), M)   # gather row p = m*H + h -> h
    mh_m = np.repeat(np.arange(M), H)  # -> m
    for k in range(NCORES):
        b0 = k * BL
        seq_k = _bf16(seq[b0 : b0 + BL].reshape(BL * C, D))
        att_k = np.ascontiguousarray(att[b0 : b0 + BL].reshape(BL * H * C, C))

        idx = np.zeros((M * H, 1 + 2 * BL), np.int32)
        for b in range(BL):
            idx[b * M : (b + 1) * M, 0] = b * C + pos[b0 + b, 0, :]
            for e in range(2):
                idx[:, 1 + b * 2 + e] = (b * H + mh_h) * C + pos[b0 + b, e, mh_m]

        in_maps.append(
            {
                "seq": seq_k,
                "attn": att_k,
                "idx": idx,
                "whT": whT,
                "wtT": wtT,
                "wbT": wbT,
                "cstf": cstf,
                "cstb": _cstb_prep(ner[b0 : b0 + BL]),
            }
        )
    return in_maps


def _get_nc():
    if "nc" not in _CACHE:
        _CACHE["nc"] = _build_nc()
    return _CACHE["nc"]


def kernel(**inputs):
    global LAST_EXEC_NS, LAST_RESULTS
    nc = _get_nc()
    in_maps = _make_in_maps(inputs)
    trace = bool(int(os.environ.get("BASS_KERNEL_TRACE", "0")))
    try:
        res = run_bass_kernel_spmd(
            nc, in_maps, core_ids=list(range(NCORES)), trace=trace
        )
    except Exception:
        if not trace:
            raise
        # tracing infra unavailable in this environment -- run untraced
        res = run_bass_kernel_spmd(
            nc, in_maps, core_ids=list(range(NCORES)), trace=False
        )
    LAST_EXEC_NS = res.exec_time_ns
    LAST_RESULTS = res
    out = np.zeros((B, NCLS), np.float32)
    for k in range(NCORES):
        out[k * BL : (k + 1) * BL] = np.asarray(res.results[k]["outT"]).T
    return out
